# revision 42
# baseline (speedup 1.0000x reference)
"""Causal self-attention with RoPE for Trainium2, 8-way SPMD.

Sharding: data-parallel over batch (2) x tensor-parallel over head-groups (4).
Core c handles batch c//4, heads 4*(c%4) .. 4*(c%4)+3.  Each core computes
q/k/v projections for its head columns of W_qkv, attention for its 4 heads,
and a partial output projection against its rows of W_proj.  The all-reduce
over the 4 cores of each batch plus b_proj happens on the host.

All device inputs ride in ONE merged [128, NF] float32 DRAM tensor per core
("big"): a small f32 section (rope tables, biases) followed by a bf16
section (x, weights, mask-mix tiles) packed two-per-f32-column and carved
with AP bitcasts.  One operand instead of fourteen keeps the per-call PJRT
dispatch cost minimal; bf16 halves the HBM traffic.

Matmul operands are bf16 (full PE rate at any moving width); accumulation,
rope arithmetic and the softmax denominator stay in f32 (PSUM / DVE).  The
partial outputs y are returned in bf16 and summed in f32 on the host.

Device layout per core:
  xT    [D=1024, T=2048]   x transposed (contraction on partitions)
  Q^T/K^T stored [128, 2, T]: tile hp holds heads {2hp, 2hp+1} (64 rows each)
  V     stored [128, NT, 4*65]: per t-block, per head 64 value cols + ones col
        (the ones column makes the PV matmul emit the softmax denominator)
  S^T   per (hp, k-block) in ONE 2-bank PSUM tile [128, 1024]: h2 half at
        cols 512*h2, so one ACT exp covers both heads of the pair.
  mask  folded into the QK accumulation: for partially-masked 128x128 blocks
        an identity matmul adds -1e5 at blocked positions (exp -> 0), keeping
        DVE off the exp->PV critical path.

PSUM budget (8 banks): 2x[128,1024] S tiles + 2x[128,1024] PV accumulators,
with the projection / output-projection chains sharing the S slots.
"""
import sys
sys.path.insert(0, "/opt/trn_rl_repo")

from contextlib import ExitStack

import ml_dtypes
import numpy as np

import concourse.bass as bass
import concourse.tile as tile
from concourse import bacc, mybir

B, T, D, H, HD = 2, 2048, 1024, 16, 64
NCORES = 8
GROUPS = 4            # tensor-parallel head groups
HPG = H // GROUPS     # heads per core
M = HPG * HD          # per-core projection width (256)
NT = T // 128         # 16 t-blocks
NQT = T // 512        # 4 q-tiles
NKT = D // 128        # 8 contraction tiles for the qkv projection
F32 = mybir.dt.float32
BF16 = mybir.dt.bfloat16
AF = mybir.ActivationFunctionType
NEG = -1.0e5          # pre-softmax logit for masked positions

# f32 section column offsets inside the merged per-core input tensor
OBIAS = 0                     # (optional) bqk [128,4] + bvb [128,M]
# bf16 section offsets (bf16 columns, from the start of the bf16 region)
BXT = 0                       # xT   [128, NKT, T]
BWQ = BXT + NKT * T           # wq   [128, NKT, M]
BWK = BWQ + NKT * M
BWV = BWK + NKT * M
BWP = BWV + NKT * M           # wp   [128, 2, D]
BVO = BWP + 2 * D             # vones [128, NT*HPG]
BC2 = BVO + NT * HPG          # c2   [128, T]
BS2 = BC2 + T                 # s2p  [128, T]
BSW = BS2 + T                 # XOR-32 permutation [128, 128]
BMM = BSW + 128               # mm [128, n_mix_cols] + ident [128, 128]


def _rope_tables():
    # mirrors reference._rope_cache in float32; s2p is sin at the swapped
    # partition (p XOR 32 within each 64-row head block), sign folded in so
    # rope is dst = raw*c2 + swap32(raw*s2p)
    inv = (1.0 / (10000.0 ** (np.arange(0, HD, 2, dtype=np.float32) / HD))).astype(np.float32)
    t = np.arange(T, dtype=np.float32)
    fr = t[:, None] * inv[None, :]                    # [T, 32]
    cos32 = np.cos(fr).T.astype(np.float32)           # [32, T]
    sin32 = np.sin(fr).T.astype(np.float32)
    c2 = np.tile(cos32, (4, 1))                                        # [128, T]
    # s2 (at destination d) = [-sin, sin, -sin, sin]; s2p[e] = s2[swap(e)]
    s2p = np.concatenate([sin32, -sin32, sin32, -sin32], 0)            # [128, T]
    return c2, s2p


def _mask_plan(mask):
    """Classify 128x128 blocks of mask^T and build the per-(q-tile, k-block)
    schedule: (lo, hi, [(col_off, mix_id), ...]) with lo/hi relative to the
    512-wide q-tile, or None when the whole block is masked out.  mix tiles
    hold NEG at blocked positions, 0 at allowed (added pre-exp via PE) and
    are deduplicated by content (a causal mask has one repeated diagonal
    pattern)."""
    mt = (np.asarray(mask).T != 0)
    nb = T // 128
    state = np.empty((nb, nb), np.int8)               # [k-block, q-block]
    for ki in range(nb):
        for qi in range(nb):
            sub = mt[128 * ki:128 * (ki + 1), 128 * qi:128 * (qi + 1)]
            state[ki, qi] = 2 if sub.all() else (1 if sub.any() else 0)
    mix_tiles = []
    mix_idx = {}

    def _mix_for(blk):
        tl = ((~blk).astype(np.float32) * NEG)
        key = tl.tobytes()
        if key not in mix_idx:
            mix_idx[key] = len(mix_tiles)
            mix_tiles.append(tl)
        return mix_idx[key]

    plan = []
    for j in range(NQT):
        row = []
        for kb in range(nb):
            sts = [state[kb, 4 * j + q] for q in range(4)]
            nz = [q for q in range(4) if sts[q] != 0]
            if not nz:
                row.append(None)
                continue
            lo_q, hi_q = nz[0], nz[-1] + 1
            mixes = []
            for q in range(lo_q, hi_q):
                if sts[q] == 1:
                    blk = mt[128 * kb:128 * (kb + 1),
                             128 * (4 * j + q):128 * (4 * j + q + 1)]
                    mixes.append((128 * q, _mix_for(blk)))
                elif sts[q] == 0:
                    # hole inside the window: NEG it out
                    mixes.append((128 * q, _mix_for(np.zeros((128, 128), bool))))
            row.append((128 * lo_q, 128 * hi_q, mixes))
        plan.append(row)
    if mix_tiles:
        mm = np.concatenate(mix_tiles, axis=1)        # [128, n_mix*128]
    else:
        mm = np.zeros((128, 128), np.float32)
    return plan, mm


def build_program(plan, n_mix_cols, dbg=False, variant=(), zero_bias=False):
    nc = bacc.Bacc("TRN2", target_bir_lowering=False, debug=False,
                   num_devices=NCORES)
    ob16 = OBIAS if zero_bias else OBIAS + 4 + M      # f32 cols before bf16
    nb16 = BMM + n_mix_cols + 128                     # total bf16 cols
    nf = ob16 + nb16 // 2
    big = nc.dram_tensor("big", [128, nf], F32, kind="ExternalInput").ap()
    bigb = big[:, ob16:nf].bitcast(BF16)              # [128, nb16]
    y = nc.dram_tensor("y", [T, D], BF16, kind="ExternalOutput").ap()

    with tile.TileContext(nc) as tc, ExitStack() as ctx:
        persist = ctx.enter_context(tc.tile_pool(name="persist", bufs=1))
        qT = persist.tile([128, 2, T], BF16, name="qT", tag="qT")
        kT = persist.tile([128, 2, T], BF16, name="kT", tag="kT")
        # 128 cols per head: col 0 = ones (softmax denominator lands on PSUM
        # partition 0, where partition_broadcast sources it without a shift
        # DMA), cols 64..127 = values (PSUM reads must start 32-aligned).
        # Cols 1..63 are never written or read; the junk they contribute to
        # PSUM partitions 1..63 is never consumed.
        v_sb = persist.tile([128, NT, HPG * 128], BF16, name="v", tag="v")
        c2_sb = persist.tile([128, T], BF16, name="c2", tag="c2")
        s2p_sb = persist.tile([128, T], BF16, name="s2p", tag="s2p")
        wp_sb = persist.tile([128, 2, D], BF16, name="wp", tag="wp")
        mm_sb = persist.tile([128, n_mix_cols], BF16, name="mm", tag="mm")
        id_sb = persist.tile([128, 128], BF16, name="ident", tag="ident")
        sw_sb = persist.tile([128, 128], BF16, name="swp", tag="swp")
        warm = persist.tile([1, 2], F32, name="warm", tag="warm")
        if not zero_bias:
            bqk_sb = persist.tile([128, 4], F32, name="bqk", tag="bqk")
            bvb_sb = persist.tile([128, M], F32, name="bvb", tag="bvb")

        pw = ctx.enter_context(tc.tile_pool(name="w", bufs=1))
        px = ctx.enter_context(tc.tile_pool(name="x", bufs=1))
        praw = ctx.enter_context(tc.tile_pool(name="praw", bufs=2))
        pcc = ctx.enter_context(tc.tile_pool(name="ropecc", bufs=2))
        pt_pool = ctx.enter_context(tc.tile_pool(name="pt", bufs=5))
        yt_pool = ctx.enter_context(tc.tile_pool(name="yt", bufs=2))
        ro_pool = ctx.enter_context(tc.tile_pool(name="ro", bufs=1))
        rb_pool = ctx.enter_context(tc.tile_pool(name="rb", bufs=1))
        out_pool = ctx.enter_context(tc.tile_pool(name="out", bufs=2))
        # PSUM: 8 banks.  psBig "s" slots are [128,1024] (2 banks) x2; the
        # S tiles, qkv projection chains and out-proj chains all rotate
        # through them.  psPV holds the two live PV accumulators (2 banks
        # each: h2 halves side by side).
        psBig = ctx.enter_context(tc.tile_pool(name="psBig", bufs=2,
                                               space="PSUM"))
        psPV = ctx.enter_context(tc.tile_pool(name="psPV", bufs=2,
                                              space="PSUM"))

        # warm up the ACT exp table while input DMAs stream
        nc.vector.memset(warm[:, 0:1], 0.0)
        nc.scalar.activation(warm[:, 1:2], warm[:, 0:1], AF.Exp)
        # zero the dead cols 1..63 of each V stationary (uninitialized SBUF
        # can hold NaN bit patterns, and NaN * 0 = NaN would poison the PSUM
        # accumulators' unread partitions); Pool is idle at startup
        nc.gpsimd.memset(
            v_sb.rearrange("p a (h e) -> p a h e", e=128)[:, :, :, 1:64], 0.0)

        # DMA order = first-use order.  xT comes in [kt, tt] tiles so the
        # first projection chain (t-chunk 0) only waits for a quarter of x.
        w_sbs = {}
        for wname in ("wk", "wq", "wv"):
            w_sbs[wname] = pw.tile([128, NKT, M], BF16, name=wname, tag=wname)
        nc.sync.dma_start(
            w_sbs["wk"], bigb[:, BWK:BWK + NKT * M].rearrange(
                "p (kt m) -> p kt m", kt=NKT))
        xT_sb = px.tile([128, NKT, T], BF16, name="xT", tag="xT")
        xTd = bigb[:, BXT:BXT + NKT * T].rearrange("p (kt t) -> p kt t",
                                                   kt=NKT)

        def load_xt(tt, eng):
            # two DMAs per t-chunk: each instruction costs ~630ns of the
            # serial HWDGE descriptor-gen stage, so batch kt in halves (the
            # halves still interleave with latency-critical rope-swap DMAs)
            cs = slice(512 * tt, 512 * (tt + 1))
            for k0 in (0, NKT // 2):
                eng.dma_start(xT_sb[:, k0:k0 + NKT // 2, cs],
                              xTd[:, k0:k0 + NKT // 2, cs])

        load_xt(0, nc.sync)
        nc.sync.dma_start(
            w_sbs["wq"], bigb[:, BWQ:BWQ + NKT * M].rearrange(
                "p (kt m) -> p kt m", kt=NKT))
        nc.sync.dma_start(c2_sb, bigb[:, BC2:BC2 + T])
        nc.sync.dma_start(s2p_sb, bigb[:, BS2:BS2 + T])
        if not zero_bias:
            nc.sync.dma_start(bqk_sb, big[:, OBIAS:OBIAS + 4])
        # swp/mm/ident before wv: the first rope phase2 needs the
        # permutation and the diagonal QK block of the first attention
        # tile needs the mask-mix tiles, and all three are tiny
        nc.sync.dma_start(sw_sb, bigb[:, BSW:BSW + 128])
        nc.sync.dma_start(mm_sb, bigb[:, BMM:BMM + n_mix_cols])
        nc.sync.dma_start(
            id_sb, bigb[:, BMM + n_mix_cols:BMM + n_mix_cols + 128])
        nc.sync.dma_start(
            w_sbs["wv"], bigb[:, BWV:BWV + NKT * M].rearrange(
                "p (kt m) -> p kt m", kt=NKT))
        if not zero_bias:
            nc.sync.dma_start(bvb_sb, big[:, OBIAS + 4:OBIAS + 4 + M])

        def load_late(j):
            # bulk loads not needed until later: issued from the SP queue
            # behind the first wave so the ACT queue carries only the exp
            # stream (a DMA holds its engine's SEQ until it clears the
            # serial HWDGE stage, which would delay exps)
            if j + 1 < NQT:
                load_xt(j + 1, nc.sync)
            if j == 0:
                # ones columns of V (the value copies only touch cols
                # 1..64/head); col 0 puts the softmax denominator on PSUM
                # partition 0, where partition_broadcast can source it
                # without a shift DMA
                nc.sync.dma_start(
                    v_sb.rearrange("p a (h e) -> p a h e", e=128)[:, :, :, 0:1],
                    bigb[:, BVO:BVO + NT * HPG].rearrange(
                        "p (a h) -> p a h", h=HPG)[:, :, :, None])
                nc.sync.dma_start(
                    wp_sb, bigb[:, BWP:BWP + 2 * D].rearrange(
                        "p (kt m) -> p kt m", kt=2))

        def qk_chunk_p1(w_sb, jt, bias_col, tt):
            """projection + the two rope products for one 512-token chunk.
            rope: dst = src*c2 + swap32(src*s2p) (sign baked into s2p).
            The 32-row partition swap runs on the PE as a permutation
            matmul (phase 2) into the unused half of this chunk's PSUM
            tile — engines cannot shift partitions and per-DMA HWDGE cost
            made the DMA variant ~630ns x 4 per chunk."""
            cs = slice(512 * tt, 512 * (tt + 1))
            ps = psBig.tile([128, 1024], F32, name="psA", tag="s")
            for kt in range(NKT):
                nc.tensor.matmul(ps[:, 0:512],
                                 w_sb[:, kt, 128 * jt:128 * (jt + 1)],
                                 xT_sb[:, kt, cs],
                                 start=(kt == 0), stop=(kt == NKT - 1))
            if zero_bias:
                src = ps[:, 0:512]
            else:
                raw = praw.tile([128, 512], F32, name="raw", tag="raw",
                                bufs=1)
                nc.scalar.activation(raw, ps[:, 0:512], AF.Identity,
                                     bias=bqk_sb[:, bias_col:bias_col + 1])
                src = raw
            u = praw.tile([128, 512], BF16, name="u", tag="u")
            cc = pcc.tile([128, 512], F32, name="cc", tag="cc")
            nc.vector.tensor_mul(u, src, s2p_sb[:, cs])
            nc.vector.tensor_mul(cc, src, c2_sb[:, cs])
            return (ps, u, cc)

        def qk_chunk_p2(state, dst, jt, tt):
            """swap matmul + final add; emitted after the NEXT chunk's
            projection so the DVE round trip for u hides under PE work."""
            ps, u, cc = state
            cs = slice(512 * tt, 512 * (tt + 1))
            nc.tensor.matmul(ps[:, 512:1024], sw_sb, u,
                             start=True, stop=True)
            nc.vector.tensor_add(dst[:, jt, cs], cc, ps[:, 512:1024])

        def v_block(tb):
            ps = psBig.tile([128, 1024], F32, name="psV", tag="s")
            for kt in range(NKT):
                nc.tensor.matmul(ps[:, 0:256],
                                 xT_sb[:, kt, 128 * tb:128 * (tb + 1)],
                                 w_sbs["wv"][:, kt, :],
                                 start=(kt == 0), stop=(kt == NKT - 1))
            vdst = v_sb[:, tb, :].rearrange("p (h e) -> p h e",
                                            e=128)[:, :, 64:128]
            psv = ps[:, 0:256].rearrange("p (h d) -> p h d", d=64)
            if zero_bias:
                nc.vector.tensor_copy(vdst, psv)
            else:
                nc.vector.tensor_add(vdst, psv,
                                     bvb_sb.rearrange("p (h d) -> p h d", d=64))

        pv_live = {}
        yt_live = {}

        def qk_block(j, hp, kb, sp):
            """QK matmuls (+ mask-bias accumulation) for one k-block."""
            lo, hi, mixes = plan[j][kb]
            for h2 in range(2):
                nc.tensor.matmul(
                    sp[:, 512 * h2 + lo:512 * h2 + hi],
                    kT[64 * h2:64 * (h2 + 1), hp, 128 * kb:128 * (kb + 1)],
                    qT[64 * h2:64 * (h2 + 1), hp,
                       512 * j + lo:512 * j + hi],
                    start=True, stop=(not mixes),
                    tile_position=(64 * h2, 0))
                # partially-masked 128-blocks: accumulate NEG at blocked
                # positions via identity matmul, exp -> 0
                for mi, (coff, mid) in enumerate(mixes):
                    nc.tensor.matmul(
                        sp[:, 512 * h2 + coff:512 * h2 + coff + 128],
                        id_sb,
                        mm_sb[:, 128 * mid:128 * (mid + 1)],
                        start=False, stop=(mi == len(mixes) - 1))

        def exp_block(j, kb, sp, pt):
            # one exp covers both h2 halves (2-bank AP)
            lo, hi, _ = plan[j][kb]
            spv = sp.rearrange("p (a q) -> p a q", a=2)[:, :, lo:hi]
            ptv = pt.rearrange("p (a q) -> p a q", a=2)[:, :, lo:hi]
            nc.scalar.activation(ptv, spv, AF.Exp, scale=1.0 / np.sqrt(HD))

        def make_emit_pv(j, hp, pv, kbs):
            started = [False, False]

            def emit_pv(kb, pt):
                lo, hi, _ = plan[j][kb]
                for h2 in range(2):
                    gh = 2 * hp + h2
                    nc.tensor.matmul(
                        pv[0:128, 512 * h2 + lo:512 * h2 + hi],
                        v_sb[:, kb, 128 * gh:128 * gh + 128],
                        pt[:, 512 * h2 + lo:512 * h2 + hi],
                        start=(not started[h2]),
                        stop=(kb == kbs[-1]))
                    started[h2] = True
            return emit_pv

        def attention_scores(j, inject=None):
            """Both head-pairs interleaved per k-block: the PE stream per
            iteration is [QK h0][QK h1][PV h0 prev][PV h1 prev] (~1.9us)
            while ACT runs [exp h0][exp h1] (~1.8us) one block behind — the
            exp round-trip latency hides entirely under the other stream's
            matmuls instead of stalling PV by ~0.6us per block.
            inject: {k_block_index: fn} — emits the next tile's K/Q
            projection chains inside this tile's attention so PE never
            drains (and its p-state stays up) across the j boundary."""
            kbs = [kb for kb in range(NT) if plan[j][kb] is not None]
            inject = inject or {}
            pvs = [psPV.tile([128, 1024], F32, name="pv", tag="pv")
                   for _ in range(2)]
            emits = [make_emit_pv(j, hp, pvs[hp], kbs) for hp in range(2)]
            pending = None        # (kb, pt_h0, pt_h1) exp'd, PV not yet
            for idx, kb in enumerate(kbs):
                fn = inject.pop(idx, None)
                if fn is not None:
                    fn()
                pts = []
                sps = []
                for hp in range(2):
                    sp = psBig.tile([128, 1024], F32, name="sp", tag="s")
                    qk_block(j, hp, kb, sp)
                    sps.append(sp)
                if pending is not None:
                    for hp in range(2):
                        emits[hp](pending[0], pending[1 + hp])
                for hp in range(2):
                    pt = pt_pool.tile([128, 1024], BF16, name="pt", tag="pt")
                    exp_block(j, kb, sps[hp], pt)
                    pts.append(pt)
                pending = (kb, *pts)
            if pending is not None:
                for hp in range(2):
                    emits[hp](pending[0], pending[1 + hp])
            for fn in inject.values():
                fn()
            pv_live[j] = pvs

        def attention_scores_mid(j, mid_hook, inject=None):
            """j=0 variant, head-pairs sequential: mid_hook is emitted inside
            hp=0 before the last k-block, after which the deferred PVs flush
            — lets the V chains land between the first QK/exp work and the
            first PV without over-subscribing the pt pool."""
            pvs_hp = []
            kbs = [kb for kb in range(NT) if plan[j][kb] is not None]
            K = len(kbs)
            inject = inject or {}
            for hp in range(2):
                pv = psPV.tile([128, 1024], F32, name="pv", tag="pv")
                emit_pv = make_emit_pv(j, hp, pv, kbs)
                if mid_hook is not None and hp == 0:
                    sps = []
                    for kb in kbs[:-1]:
                        sp = psBig.tile([128, 1024], F32, name="sp", tag="s")
                        pt = pt_pool.tile([128, 1024], BF16, name="pt",
                                          tag="pt")
                        qk_block(j, hp, kb, sp)
                        exp_block(j, kb, sp, pt)
                        sps.append((kb, pt))
                    mid_hook()
                    mid_hook = None
                    kb = kbs[-1]
                    sp = psBig.tile([128, 1024], F32, name="sp", tag="s")
                    qk_block(j, hp, kb, sp)
                    for pkb, ppt in sps:
                        emit_pv(pkb, ppt)
                    pt = pt_pool.tile([128, 1024], BF16, name="pt", tag="pt")
                    exp_block(j, kb, sp, pt)
                    emit_pv(kb, pt)
                    pvs_hp.append(pv)
                    continue
                pending = None            # (kb, pt) with exp done, PV not yet
                for idx, kb in enumerate(kbs):
                    fn = inject.pop(hp * K + idx, None)
                    if fn is not None:
                        fn()
                    sp = psBig.tile([128, 1024], F32, name="sp", tag="s")
                    pt = pt_pool.tile([128, 1024], BF16, name="pt", tag="pt")
                    qk_block(j, hp, kb, sp)
                    if pending is not None:
                        emit_pv(*pending)
                    exp_block(j, kb, sp, pt)
                    pending = (kb, pt)
                if pending is not None:
                    emit_pv(*pending)
                pvs_hp.append(pv)
            for fn in inject.values():
                fn()
            pv_live[j] = pvs_hp

        def finish_normalize(j):
            yts = []
            pvs_hp = pv_live.pop(j)
            for hp in range(2):
                pv = pvs_hp[hp]
                yt = yt_pool.tile([128, 512], BF16, name="yt", tag="yt")
                # denominators (both h2) sit on psum partition 0
                r = ro_pool.tile([1, 1024], F32, name="r", tag="r")
                nc.vector.reciprocal(r[0:1, :], pv[0:1, :])
                rb = rb_pool.tile([64, 1024], F32, name="rb", tag="rb")
                nc.gpsimd.partition_broadcast(rb, r[0:1, :])
                nc.vector.tensor_mul(yt[0:64, :], pv[64:128, 0:512],
                                     rb[:, 0:512])
                # rows 64..127: compute at 0..63 then DMA-shift
                tmp = rb_pool.tile([64, 512], BF16, name="tmp", tag="tmp")
                nc.vector.tensor_mul(tmp, pv[64:128, 512:1024],
                                     rb[:, 512:1024])
                nc.sync.dma_start(yt[64:128, :], tmp)
                yts.append(yt)
            yt_live[j] = yts

        def finish_outproj(j):
            yts = yt_live.pop(j)
            for tb in range(4):
                po = psBig.tile([128, 1024], F32, name="po", tag="s")
                for nn in range(2):
                    for kt2 in range(2):
                        nc.tensor.matmul(
                            po[:, 512 * nn:512 * (nn + 1)],
                            yts[kt2][:, 128 * tb:128 * (tb + 1)],
                            wp_sb[:, kt2, 512 * nn:512 * (nn + 1)],
                            start=(kt2 == 0), stop=(kt2 == 1))
                ob = out_pool.tile([128, D], BF16, name="ob", tag="ob")
                nc.vector.tensor_copy(ob, po)
                nc.sync.dma_start(
                    y[512 * j + 128 * tb:512 * j + 128 * (tb + 1), :], ob)

        # Emission order per q-tile j (engines run their streams IN ORDER):
        #   [K/Q chains j][V chains j][scores j][out-proj j-1][normalize j]
        # The out-proj of j-1 sits AFTER scores(j) in the PE stream so PE
        # never stalls on j-1's normalize (which drains on DVE/Pool/DMA
        # while the scores matmuls run).  For j=0 the V chains are hoisted
        # into the middle of the hp=0 scores via mid_hook so the first
        # exp fires as soon as K/Q chunk 0 exists.
        produced_kq = [-1]
        produced_v = [-1]

        def emit_k_chains(t):
            s0 = qk_chunk_p1(w_sbs["wk"], 0, 2, t)
            s1 = qk_chunk_p1(w_sbs["wk"], 1, 3, t)
            qk_chunk_p2(s0, kT, 0, t)
            qk_chunk_p2(s1, kT, 1, t)

        def emit_q_chains(t):
            s0 = qk_chunk_p1(w_sbs["wq"], 0, 0, t)
            s1 = qk_chunk_p1(w_sbs["wq"], 1, 1, t)
            qk_chunk_p2(s0, qT, 0, t)
            qk_chunk_p2(s1, qT, 1, t)
            produced_kq[0] = t

        def ensure_kq(t):
            # startup path: pipeline phase2 of each chunk behind the next
            # chunk's projection so the u-product round trip on DVE never
            # stalls the PE
            for tt in range(produced_kq[0] + 1, t + 1):
                sk0 = qk_chunk_p1(w_sbs["wk"], 0, 2, tt)
                sk1 = qk_chunk_p1(w_sbs["wk"], 1, 3, tt)
                qk_chunk_p2(sk0, kT, 0, tt)
                sq0 = qk_chunk_p1(w_sbs["wq"], 0, 0, tt)
                qk_chunk_p2(sk1, kT, 1, tt)
                sq1 = qk_chunk_p1(w_sbs["wq"], 1, 1, tt)
                qk_chunk_p2(sq0, qT, 0, tt)
                qk_chunk_p2(sq1, qT, 1, tt)
                produced_kq[0] = tt

        def ensure_v(t):
            for tt in range(produced_v[0] + 1, t + 1):
                for tb in range(4 * tt, 4 * tt + 4):
                    v_block(tb)
                produced_v[0] = tt

        def needed_v(j):
            kbs = [kb for kb in range(NT) if plan[j][kb] is not None]
            return max(kb // 4 for kb in kbs) if kbs else 0

        for j in range(NQT):
            ensure_kq(j)
            load_late(j)
            use_mid = (j == 0 and needed_v(0) == 0)
            inject = {}
            if j + 1 < NQT:
                # emit the next tile's K/Q chains inside this tile's
                # attention tail: their rope latency hides under the last
                # QK/exp/PV steps instead of stalling the j+1 boundary
                nkb = len([kb for kb in range(NT) if plan[j][kb] is not None])
                t_next = j + 1
                if use_mid:          # flat (hp*K + idx) steps
                    ik = max(nkb + 1, 2 * nkb - 5)
                    iq = max(nkb + 2, 2 * nkb - 3)
                else:                # per-k-block steps
                    ik = max(1, nkb - 3)
                    iq = max(2, nkb - 2)
                inject[ik] = lambda t=t_next: emit_k_chains(t)
                inject[iq] = lambda t=t_next: emit_q_chains(t)
            if use_mid:
                attention_scores_mid(0, mid_hook=lambda: ensure_v(0),
                                     inject=inject)
            else:
                ensure_v(max(needed_v(j), j))
                attention_scores(j, inject=inject)
            if j > 0:
                finish_outproj(j - 1)
            finish_normalize(j)
        finish_outproj(NQT - 1)
    nc.finalize()
    return nc


def make_core_inputs(x, mask, W_qkv, b_qkv, W_proj, b_proj, mm,
                     zero_bias=False):
    """Per-core input dicts: one merged [128, NF] float32 array per core."""
    x = np.asarray(x, np.float32)
    W_qkv = np.asarray(W_qkv, np.float32)
    b_qkv = np.asarray(b_qkv, np.float32)
    W_proj = np.asarray(W_proj, np.float32)
    c2, s2p = _rope_tables()

    def fold128(w):                       # [(kt*128), m] -> [128, kt*m]
        kt = w.shape[0] // 128
        return np.ascontiguousarray(
            w.reshape(kt, 128, w.shape[1]).transpose(1, 0, 2).reshape(
                128, kt * w.shape[1]))

    def pack16(pieces):                   # bf16 [128, c] list -> f32 [128, C/2]
        cat = np.concatenate(
            [np.asarray(p, dtype=ml_dtypes.bfloat16) for p in pieces], axis=1)
        return cat.view(np.uint16).view(np.float32)

    vones = np.ones((128, NT * HPG), np.float32)
    ident = np.eye(128, dtype=np.float32)
    swp = ident[[r ^ 32 for r in range(128)]]         # XOR-32 permutation
    in_maps = []
    for c in range(NCORES):
        b, g = divmod(c, GROUPS)
        xT = np.ascontiguousarray(x[b].T)
        sl = slice(M * g, M * (g + 1))
        f32_pieces = []
        if not zero_bias:
            bq = b_qkv[0 * D:1 * D][sl]
            bk = b_qkv[1 * D:2 * D][sl]
            bv = b_qkv[2 * D:3 * D][sl]
            bqk = np.stack([bq[0:128], bq[128:256],
                            bk[0:128], bk[128:256]], axis=1)
            f32_pieces.append(bqk.astype(np.float32))
            f32_pieces.append(np.tile(bv[None, :], (128, 1)).astype(np.float32))
        b16 = pack16([
            fold128(xT),
            fold128(W_qkv[:, 0 * D:1 * D][:, sl]),
            fold128(W_qkv[:, 1 * D:2 * D][:, sl]),
            fold128(W_qkv[:, 2 * D:3 * D][:, sl]),
            fold128(W_proj[sl, :]),
            vones, c2, s2p, swp, mm, ident,
        ])
        f32_pieces.append(b16)
        in_maps.append({"big": np.ascontiguousarray(
            np.concatenate(f32_pieces, axis=1), dtype=np.float32)})
    return in_maps


def gather_output(results, b_proj):
    out = np.zeros((B, T, D), np.float32)
    for c in range(NCORES):
        b = c // GROUPS
        out[b] += np.asarray(results[c]["y"], dtype=np.float32)
    out += np.asarray(b_proj, np.float32)[None, None, :]
    return out


def kernel(x, mask, W_qkv, b_qkv, W_proj, b_proj):
    from concourse.bass_utils import run_bass_kernel_spmd
    plan, mm = _mask_plan(mask)
    zb = not (np.any(np.asarray(b_qkv)))
    nc = build_program(plan, mm.shape[1], zero_bias=zb)
    in_maps = make_core_inputs(x, mask, W_qkv, b_qkv, W_proj, b_proj, mm,
                               zero_bias=zb)
    res = run_bass_kernel_spmd(nc, in_maps, list(range(NCORES)))
    return gather_output(res.results, b_proj)


# revision 56
# speedup vs baseline: 1.2774x; 1.2774x over previous
"""Causal self-attention with RoPE for Trainium2, 8-way SPMD.

Sharding: data-parallel over batch (2) x tensor-parallel over head-groups (4).
Core c handles batch c//4, heads 4*(c%4) .. 4*(c%4)+3.  Each core computes
q/k/v projections for its head columns of W_qkv, attention for its 4 heads,
and a partial output projection against its rows of W_proj.  The all-reduce
over the 4 cores of each batch plus b_proj happens on the host.

All device inputs ride in ONE merged [128, NF] float32 DRAM tensor per core
("big"): a small f32 section (rope tables, biases) followed by a bf16
section (x, weights, mask-mix tiles) packed two-per-f32-column and carved
with AP bitcasts.  One operand instead of fourteen keeps the per-call PJRT
dispatch cost minimal; bf16 halves the HBM traffic.

Matmul operands are bf16 (full PE rate at any moving width); accumulation,
rope arithmetic and the softmax denominator stay in f32 (PSUM / DVE).  The
partial outputs y are returned in bf16 and summed in f32 on the host.

Device layout per core:
  xT    [D=1024, T=2048]   x transposed (contraction on partitions)
  Q^T/K^T stored [128, 2, T]: tile hp holds heads {2hp, 2hp+1} (64 rows each)
  V     stored [128, NT, 4*65]: per t-block, per head 64 value cols + ones col
        (the ones column makes the PV matmul emit the softmax denominator)
  S^T   per (hp, k-block) in ONE 2-bank PSUM tile [128, 1024]: h2 half at
        cols 512*h2, so one ACT exp covers both heads of the pair.
  mask  folded into the QK accumulation: for partially-masked 128x128 blocks
        an identity matmul adds -1e5 at blocked positions (exp -> 0), keeping
        DVE off the exp->PV critical path.

PSUM budget (8 banks): 2x[128,1024] S tiles + 2x[128,1024] PV accumulators,
with the projection / output-projection chains sharing the S slots.
"""
import sys
sys.path.insert(0, "/opt/trn_rl_repo")

from contextlib import ExitStack

import ml_dtypes
import numpy as np

import concourse.bass as bass
import concourse.tile as tile
from concourse import bacc, mybir

B, T, D, H, HD = 2, 2048, 1024, 16, 64
NCORES = 4            # fewer cores = cheaper per-call PJRT dispatch; the
                      # doubled per-core work still hides under the
                      # transport pipeline
GROUPS = 4            # tensor-parallel head groups (one per core; each
                      # core processes BOTH batches sequentially)
HPG = H // GROUPS     # heads per core
M = HPG * HD          # per-core projection width (256)
NT = T // 128         # 16 t-blocks
NQT = T // 512        # 4 q-tiles
NKT = D // 128        # 8 contraction tiles for the qkv projection
F32 = mybir.dt.float32
BF16 = mybir.dt.bfloat16
AF = mybir.ActivationFunctionType
NEG = -1.0e5          # pre-softmax logit for masked positions

# f32 section column offsets inside the merged per-core input tensor
OBIAS = 0                     # (optional) bqk [128,4] + bvb [128,M]
# bf16 section offsets (bf16 columns, from the start of the bf16 region)
BXT = 0                       # xT   [128, B*NKT, T]
BWQ = BXT + B * NKT * T       # wq   [128, NKT, M]
BWK = BWQ + NKT * M
BWV = BWK + NKT * M
BWP = BWV + NKT * M           # wp   [128, 2, D]
BVO = BWP + 2 * D             # vones [128, NT*HPG]
BC2 = BVO + NT * HPG          # c2   [128, T]
BS2 = BC2 + T                 # s2p  [128, T]
BSW = BS2 + T                 # XOR-32 permutation [128, 128]
BMM = BSW + 128               # mm [128, n_mix_cols] + ident [128, 128]


def _rope_tables():
    # mirrors reference._rope_cache in float32; s2p is sin at the swapped
    # partition (p XOR 32 within each 64-row head block), sign folded in so
    # rope is dst = raw*c2 + swap32(raw*s2p)
    inv = (1.0 / (10000.0 ** (np.arange(0, HD, 2, dtype=np.float32) / HD))).astype(np.float32)
    t = np.arange(T, dtype=np.float32)
    fr = t[:, None] * inv[None, :]                    # [T, 32]
    cos32 = np.cos(fr).T.astype(np.float32)           # [32, T]
    sin32 = np.sin(fr).T.astype(np.float32)
    c2 = np.tile(cos32, (4, 1))                                        # [128, T]
    # s2 (at destination d) = [-sin, sin, -sin, sin]; s2p[e] = s2[swap(e)]
    s2p = np.concatenate([sin32, -sin32, sin32, -sin32], 0)            # [128, T]
    return c2, s2p


def _mask_plan(mask):
    """Classify 128x128 blocks of mask^T and build the per-(q-tile, k-block)
    schedule: (lo, hi, [(col_off, mix_id), ...]) with lo/hi relative to the
    512-wide q-tile, or None when the whole block is masked out.  mix tiles
    hold NEG at blocked positions, 0 at allowed (added pre-exp via PE) and
    are deduplicated by content (a causal mask has one repeated diagonal
    pattern)."""
    mt = (np.asarray(mask).T != 0)
    nb = T // 128
    state = np.empty((nb, nb), np.int8)               # [k-block, q-block]
    for ki in range(nb):
        for qi in range(nb):
            sub = mt[128 * ki:128 * (ki + 1), 128 * qi:128 * (qi + 1)]
            state[ki, qi] = 2 if sub.all() else (1 if sub.any() else 0)
    mix_tiles = []
    mix_idx = {}

    def _mix_for(blk):
        tl = ((~blk).astype(np.float32) * NEG)
        key = tl.tobytes()
        if key not in mix_idx:
            mix_idx[key] = len(mix_tiles)
            mix_tiles.append(tl)
        return mix_idx[key]

    plan = []
    for j in range(NQT):
        row = []
        for kb in range(nb):
            sts = [state[kb, 4 * j + q] for q in range(4)]
            nz = [q for q in range(4) if sts[q] != 0]
            if not nz:
                row.append(None)
                continue
            lo_q, hi_q = nz[0], nz[-1] + 1
            mixes = []
            for q in range(lo_q, hi_q):
                if sts[q] == 1:
                    blk = mt[128 * kb:128 * (kb + 1),
                             128 * (4 * j + q):128 * (4 * j + q + 1)]
                    mixes.append((128 * q, _mix_for(blk)))
                elif sts[q] == 0:
                    # hole inside the window: NEG it out
                    mixes.append((128 * q, _mix_for(np.zeros((128, 128), bool))))
            row.append((128 * lo_q, 128 * hi_q, mixes))
        plan.append(row)
    if mix_tiles:
        mm = np.concatenate(mix_tiles, axis=1)        # [128, n_mix*128]
    else:
        mm = np.zeros((128, 128), np.float32)
    return plan, mm


def build_program(plan, n_mix_cols, dbg=False, variant=(), zero_bias=False):
    nc = bacc.Bacc("TRN2", target_bir_lowering=False, debug=False,
                   num_devices=NCORES)
    ob16 = OBIAS if zero_bias else OBIAS + 4 + M      # f32 cols before bf16
    nb16 = BMM + n_mix_cols + 128                     # total bf16 cols
    nf = ob16 + nb16 // 2
    big = nc.dram_tensor("big", [128, nf], F32, kind="ExternalInput").ap()
    bigb = big[:, ob16:nf].bitcast(BF16)              # [128, nb16]
    y = nc.dram_tensor("y", [B * T, D], BF16, kind="ExternalOutput").ap()

    with tile.TileContext(nc) as tc, ExitStack() as ctx:
        persist = ctx.enter_context(tc.tile_pool(name="persist", bufs=1))
        qT = persist.tile([128, 2, T], BF16, name="qT", tag="qT")
        kT = persist.tile([128, 2, T], BF16, name="kT", tag="kT")
        # 128 cols per head: col 0 = ones (softmax denominator lands on PSUM
        # partition 0, where partition_broadcast sources it without a shift
        # DMA), cols 64..127 = values (PSUM reads must start 32-aligned).
        # Cols 1..63 are never written or read; the junk they contribute to
        # PSUM partitions 1..63 is never consumed.
        v_sb = persist.tile([128, NT, HPG * 128], BF16, name="v", tag="v")
        c2_sb = persist.tile([128, T], BF16, name="c2", tag="c2")
        s2p_sb = persist.tile([128, T], BF16, name="s2p", tag="s2p")
        wp_sb = persist.tile([128, 2, D], BF16, name="wp", tag="wp")
        mm_sb = persist.tile([128, n_mix_cols], BF16, name="mm", tag="mm")
        id_sb = persist.tile([128, 128], BF16, name="ident", tag="ident")
        sw_sb = persist.tile([128, 128], BF16, name="swp", tag="swp")
        warm = persist.tile([1, 2], F32, name="warm", tag="warm")
        if not zero_bias:
            bqk_sb = persist.tile([128, 4], F32, name="bqk", tag="bqk")
            bvb_sb = persist.tile([128, M], F32, name="bvb", tag="bvb")

        pw = ctx.enter_context(tc.tile_pool(name="w", bufs=1))
        px = ctx.enter_context(tc.tile_pool(name="x", bufs=1))
        praw = ctx.enter_context(tc.tile_pool(name="praw", bufs=2))
        pcc = ctx.enter_context(tc.tile_pool(name="ropecc", bufs=2))
        pt_pool = ctx.enter_context(tc.tile_pool(name="pt", bufs=5))
        yt_pool = ctx.enter_context(tc.tile_pool(name="yt", bufs=2))
        ro_pool = ctx.enter_context(tc.tile_pool(name="ro", bufs=1))
        rb_pool = ctx.enter_context(tc.tile_pool(name="rb", bufs=1))
        out_pool = ctx.enter_context(tc.tile_pool(name="out", bufs=2))
        # PSUM: 8 banks.  psBig "s" slots are [128,1024] (2 banks) x2; the
        # S tiles, qkv projection chains and out-proj chains all rotate
        # through them.  psPV holds the two live PV accumulators (2 banks
        # each: h2 halves side by side).
        psBig = ctx.enter_context(tc.tile_pool(name="psBig", bufs=2,
                                               space="PSUM"))
        psPV = ctx.enter_context(tc.tile_pool(name="psPV", bufs=2,
                                              space="PSUM"))

        # warm up the ACT exp table while input DMAs stream
        nc.vector.memset(warm[:, 0:1], 0.0)
        nc.scalar.activation(warm[:, 1:2], warm[:, 0:1], AF.Exp)
        # zero the dead cols 1..63 of each V stationary (uninitialized SBUF
        # can hold NaN bit patterns, and NaN * 0 = NaN would poison the PSUM
        # accumulators' unread partitions); Pool is idle at startup
        nc.gpsimd.memset(
            v_sb.rearrange("p a (h e) -> p a h e", e=128)[:, :, :, 1:64], 0.0)

        # DMA order = first-use order.  xT comes in [kt, tt] tiles so the
        # first projection chain (t-chunk 0) only waits for a quarter of x.
        w_sbs = {}
        for wname in ("wk", "wq", "wv"):
            w_sbs[wname] = pw.tile([128, NKT, M], BF16, name=wname, tag=wname)
        nc.sync.dma_start(
            w_sbs["wk"], bigb[:, BWK:BWK + NKT * M].rearrange(
                "p (kt m) -> p kt m", kt=NKT))
        xT_sb = px.tile([128, B * NKT, T], BF16, name="xT", tag="xT")
        xTd = bigb[:, BXT:BXT + B * NKT * T].rearrange(
            "p (kt t) -> p kt t", kt=B * NKT)

        def load_xt(b, tt, eng):
            # two DMAs per t-chunk: each instruction costs ~630ns of the
            # serial HWDGE descriptor-gen stage, so batch kt in halves
            cs = slice(512 * tt, 512 * (tt + 1))
            for k0 in (b * NKT, b * NKT + NKT // 2):
                eng.dma_start(xT_sb[:, k0:k0 + NKT // 2, cs],
                              xTd[:, k0:k0 + NKT // 2, cs])

        load_xt(0, 0, nc.sync)
        nc.sync.dma_start(
            w_sbs["wq"], bigb[:, BWQ:BWQ + NKT * M].rearrange(
                "p (kt m) -> p kt m", kt=NKT))
        nc.sync.dma_start(c2_sb, bigb[:, BC2:BC2 + T])
        nc.sync.dma_start(s2p_sb, bigb[:, BS2:BS2 + T])
        if not zero_bias:
            nc.sync.dma_start(bqk_sb, big[:, OBIAS:OBIAS + 4])
        # swp/mm/ident before wv: the first rope phase2 needs the
        # permutation and the diagonal QK block of the first attention
        # tile needs the mask-mix tiles, and all three are tiny
        nc.sync.dma_start(sw_sb, bigb[:, BSW:BSW + 128])
        nc.sync.dma_start(mm_sb, bigb[:, BMM:BMM + n_mix_cols])
        nc.sync.dma_start(
            id_sb, bigb[:, BMM + n_mix_cols:BMM + n_mix_cols + 128])
        nc.sync.dma_start(
            w_sbs["wv"], bigb[:, BWV:BWV + NKT * M].rearrange(
                "p (kt m) -> p kt m", kt=NKT))
        if not zero_bias:
            nc.sync.dma_start(bvb_sb, big[:, OBIAS + 4:OBIAS + 4 + M])

        def load_late(b, j):
            # bulk loads not needed until later: issued from the SP queue
            # behind the first wave so the ACT queue carries only the exp
            # stream (a DMA holds its engine's SEQ until it clears the
            # serial HWDGE stage, which would delay exps)
            flat = b * NQT + j + 1
            if flat < B * NQT:
                load_xt(flat // NQT, flat % NQT, nc.sync)
            if b == 0 and j == 0:
                # ones columns of V (the value copies only touch cols
                # 1..64/head); col 0 puts the softmax denominator on PSUM
                # partition 0, where partition_broadcast can source it
                # without a shift DMA
                nc.sync.dma_start(
                    v_sb.rearrange("p a (h e) -> p a h e", e=128)[:, :, :, 0:1],
                    bigb[:, BVO:BVO + NT * HPG].rearrange(
                        "p (a h) -> p a h", h=HPG)[:, :, :, None])
                nc.sync.dma_start(
                    wp_sb, bigb[:, BWP:BWP + 2 * D].rearrange(
                        "p (kt m) -> p kt m", kt=2))

        def qk_chunk_p1(b, w_sb, jt, bias_col, tt):
            """projection + the two rope products for one 512-token chunk.
            rope: dst = src*c2 + swap32(src*s2p) (sign baked into s2p).
            The 32-row partition swap runs on the PE as a permutation
            matmul (phase 2) into the unused half of this chunk's PSUM
            tile — engines cannot shift partitions and per-DMA HWDGE cost
            made the DMA variant ~630ns x 4 per chunk."""
            cs = slice(512 * tt, 512 * (tt + 1))
            ps = psBig.tile([128, 1024], F32, name="psA", tag="s")
            for kt in range(NKT):
                nc.tensor.matmul(ps[:, 0:512],
                                 w_sb[:, kt, 128 * jt:128 * (jt + 1)],
                                 xT_sb[:, b * NKT + kt, cs],
                                 start=(kt == 0), stop=(kt == NKT - 1))
            if zero_bias:
                src = ps[:, 0:512]
            else:
                raw = praw.tile([128, 512], F32, name="raw", tag="raw",
                                bufs=1)
                nc.scalar.activation(raw, ps[:, 0:512], AF.Identity,
                                     bias=bqk_sb[:, bias_col:bias_col + 1])
                src = raw
            u = praw.tile([128, 512], BF16, name="u", tag="u")
            cc = pcc.tile([128, 512], F32, name="cc", tag="cc")
            nc.vector.tensor_mul(u, src, s2p_sb[:, cs])
            nc.vector.tensor_mul(cc, src, c2_sb[:, cs])
            return (ps, u, cc)

        def qk_chunk_p2(state, dst, jt, tt):
            """swap matmul + final add; emitted after the NEXT chunk's
            projection so the DVE round trip for u hides under PE work."""
            ps, u, cc = state
            cs = slice(512 * tt, 512 * (tt + 1))
            nc.tensor.matmul(ps[:, 512:1024], sw_sb, u,
                             start=True, stop=True)
            nc.vector.tensor_add(dst[:, jt, cs], cc, ps[:, 512:1024])

        def v_block(b, tb):
            ps = psBig.tile([128, 1024], F32, name="psV", tag="s")
            for kt in range(NKT):
                nc.tensor.matmul(ps[:, 0:256],
                                 xT_sb[:, b * NKT + kt, 128 * tb:128 * (tb + 1)],
                                 w_sbs["wv"][:, kt, :],
                                 start=(kt == 0), stop=(kt == NKT - 1))
            vdst = v_sb[:, tb, :].rearrange("p (h e) -> p h e",
                                            e=128)[:, :, 64:128]
            psv = ps[:, 0:256].rearrange("p (h d) -> p h d", d=64)
            if zero_bias:
                nc.vector.tensor_copy(vdst, psv)
            else:
                nc.vector.tensor_add(vdst, psv,
                                     bvb_sb.rearrange("p (h d) -> p h d", d=64))

        pv_live = {}
        yt_live = {}

        def qk_block(j, hp, kb, sp):
            """QK matmuls (+ mask-bias accumulation) for one k-block."""
            lo, hi, mixes = plan[j][kb]
            for h2 in range(2):
                nc.tensor.matmul(
                    sp[:, 512 * h2 + lo:512 * h2 + hi],
                    kT[64 * h2:64 * (h2 + 1), hp, 128 * kb:128 * (kb + 1)],
                    qT[64 * h2:64 * (h2 + 1), hp,
                       512 * j + lo:512 * j + hi],
                    start=True, stop=(not mixes),
                    tile_position=(64 * h2, 0))
                # partially-masked 128-blocks: accumulate NEG at blocked
                # positions via identity matmul, exp -> 0
                for mi, (coff, mid) in enumerate(mixes):
                    nc.tensor.matmul(
                        sp[:, 512 * h2 + coff:512 * h2 + coff + 128],
                        id_sb,
                        mm_sb[:, 128 * mid:128 * (mid + 1)],
                        start=False, stop=(mi == len(mixes) - 1))

        def exp_block(j, kb, sp, pt):
            # one exp covers both h2 halves (2-bank AP)
            lo, hi, _ = plan[j][kb]
            spv = sp.rearrange("p (a q) -> p a q", a=2)[:, :, lo:hi]
            ptv = pt.rearrange("p (a q) -> p a q", a=2)[:, :, lo:hi]
            nc.scalar.activation(ptv, spv, AF.Exp, scale=1.0 / np.sqrt(HD))

        def make_emit_pv(j, hp, pv, kbs):
            started = [False, False]

            def emit_pv(kb, pt):
                lo, hi, _ = plan[j][kb]
                for h2 in range(2):
                    gh = 2 * hp + h2
                    nc.tensor.matmul(
                        pv[0:128, 512 * h2 + lo:512 * h2 + hi],
                        v_sb[:, kb, 128 * gh:128 * gh + 128],
                        pt[:, 512 * h2 + lo:512 * h2 + hi],
                        start=(not started[h2]),
                        stop=(kb == kbs[-1]))
                    started[h2] = True
            return emit_pv

        def attention_scores(j, inject=None):
            """Both head-pairs interleaved per k-block: the PE stream per
            iteration is [QK h0][QK h1][PV h0 prev][PV h1 prev] (~1.9us)
            while ACT runs [exp h0][exp h1] (~1.8us) one block behind — the
            exp round-trip latency hides entirely under the other stream's
            matmuls instead of stalling PV by ~0.6us per block.
            inject: {k_block_index: fn} — emits the next tile's K/Q
            projection chains inside this tile's attention so PE never
            drains (and its p-state stays up) across the j boundary."""
            kbs = [kb for kb in range(NT) if plan[j][kb] is not None]
            inject = inject or {}
            pvs = [psPV.tile([128, 1024], F32, name="pv", tag="pv")
                   for _ in range(2)]
            emits = [make_emit_pv(j, hp, pvs[hp], kbs) for hp in range(2)]
            pending = None        # (kb, pt_h0, pt_h1) exp'd, PV not yet
            for idx, kb in enumerate(kbs):
                fn = inject.pop(idx, None)
                if fn is not None:
                    fn()
                pts = []
                sps = []
                for hp in range(2):
                    sp = psBig.tile([128, 1024], F32, name="sp", tag="s")
                    qk_block(j, hp, kb, sp)
                    sps.append(sp)
                if pending is not None:
                    for hp in range(2):
                        emits[hp](pending[0], pending[1 + hp])
                for hp in range(2):
                    pt = pt_pool.tile([128, 1024], BF16, name="pt", tag="pt")
                    exp_block(j, kb, sps[hp], pt)
                    pts.append(pt)
                pending = (kb, *pts)
            if pending is not None:
                for hp in range(2):
                    emits[hp](pending[0], pending[1 + hp])
            for fn in inject.values():
                fn()
            pv_live[j] = pvs

        def attention_scores_mid(j, mid_hook, inject=None):
            """j=0 variant, head-pairs sequential: mid_hook is emitted inside
            hp=0 before the last k-block, after which the deferred PVs flush
            — lets the V chains land between the first QK/exp work and the
            first PV without over-subscribing the pt pool."""
            pvs_hp = []
            kbs = [kb for kb in range(NT) if plan[j][kb] is not None]
            K = len(kbs)
            inject = inject or {}
            for hp in range(2):
                pv = psPV.tile([128, 1024], F32, name="pv", tag="pv")
                emit_pv = make_emit_pv(j, hp, pv, kbs)
                if mid_hook is not None and hp == 0:
                    sps = []
                    for kb in kbs[:-1]:
                        sp = psBig.tile([128, 1024], F32, name="sp", tag="s")
                        pt = pt_pool.tile([128, 1024], BF16, name="pt",
                                          tag="pt")
                        qk_block(j, hp, kb, sp)
                        exp_block(j, kb, sp, pt)
                        sps.append((kb, pt))
                    mid_hook()
                    mid_hook = None
                    kb = kbs[-1]
                    sp = psBig.tile([128, 1024], F32, name="sp", tag="s")
                    qk_block(j, hp, kb, sp)
                    for pkb, ppt in sps:
                        emit_pv(pkb, ppt)
                    pt = pt_pool.tile([128, 1024], BF16, name="pt", tag="pt")
                    exp_block(j, kb, sp, pt)
                    emit_pv(kb, pt)
                    pvs_hp.append(pv)
                    continue
                pending = None            # (kb, pt) with exp done, PV not yet
                for idx, kb in enumerate(kbs):
                    fn = inject.pop(hp * K + idx, None)
                    if fn is not None:
                        fn()
                    sp = psBig.tile([128, 1024], F32, name="sp", tag="s")
                    pt = pt_pool.tile([128, 1024], BF16, name="pt", tag="pt")
                    qk_block(j, hp, kb, sp)
                    if pending is not None:
                        emit_pv(*pending)
                    exp_block(j, kb, sp, pt)
                    pending = (kb, pt)
                if pending is not None:
                    emit_pv(*pending)
                pvs_hp.append(pv)
            for fn in inject.values():
                fn()
            pv_live[j] = pvs_hp

        def finish_normalize(j):
            yts = []
            pvs_hp = pv_live.pop(j)
            for hp in range(2):
                pv = pvs_hp[hp]
                yt = yt_pool.tile([128, 512], BF16, name="yt", tag="yt")
                # denominators (both h2) sit on psum partition 0
                r = ro_pool.tile([1, 1024], F32, name="r", tag="r")
                nc.vector.reciprocal(r[0:1, :], pv[0:1, :])
                rb = rb_pool.tile([64, 1024], F32, name="rb", tag="rb")
                nc.gpsimd.partition_broadcast(rb, r[0:1, :])
                nc.vector.tensor_mul(yt[0:64, :], pv[64:128, 0:512],
                                     rb[:, 0:512])
                # rows 64..127: compute at 0..63 then DMA-shift
                tmp = rb_pool.tile([64, 512], BF16, name="tmp", tag="tmp")
                nc.vector.tensor_mul(tmp, pv[64:128, 512:1024],
                                     rb[:, 512:1024])
                nc.sync.dma_start(yt[64:128, :], tmp)
                yts.append(yt)
            yt_live[j] = yts

        def finish_outproj(b, j):
            yts = yt_live.pop(j)
            for tb in range(4):
                po = psBig.tile([128, 1024], F32, name="po", tag="s")
                for nn in range(2):
                    for kt2 in range(2):
                        nc.tensor.matmul(
                            po[:, 512 * nn:512 * (nn + 1)],
                            yts[kt2][:, 128 * tb:128 * (tb + 1)],
                            wp_sb[:, kt2, 512 * nn:512 * (nn + 1)],
                            start=(kt2 == 0), stop=(kt2 == 1))
                ob = out_pool.tile([128, D], BF16, name="ob", tag="ob")
                nc.vector.tensor_copy(ob, po)
                r0 = b * T + 512 * j + 128 * tb
                nc.sync.dma_start(y[r0:r0 + 128, :], ob)

        # Emission order per q-tile j (engines run their streams IN ORDER):
        #   [K/Q chains j][V chains j][scores j][out-proj j-1][normalize j]
        # The out-proj of j-1 sits AFTER scores(j) in the PE stream so PE
        # never stalls on j-1's normalize (which drains on DVE/Pool/DMA
        # while the scores matmuls run).  For j=0 the V chains are hoisted
        # into the middle of the hp=0 scores via mid_hook so the first
        # exp fires as soon as K/Q chunk 0 exists.
        produced_kq = [-1]
        produced_v = [-1]

        def emit_k_chains(b, t):
            s0 = qk_chunk_p1(b, w_sbs["wk"], 0, 2, t)
            s1 = qk_chunk_p1(b, w_sbs["wk"], 1, 3, t)
            qk_chunk_p2(s0, kT, 0, t)
            qk_chunk_p2(s1, kT, 1, t)

        def emit_q_chains(b, t):
            s0 = qk_chunk_p1(b, w_sbs["wq"], 0, 0, t)
            s1 = qk_chunk_p1(b, w_sbs["wq"], 1, 1, t)
            qk_chunk_p2(s0, qT, 0, t)
            qk_chunk_p2(s1, qT, 1, t)
            produced_kq[0] = t

        def ensure_kq(b, t):
            # startup path: pipeline phase2 of each chunk behind the next
            # chunk's projection so the u-product round trip on DVE never
            # stalls the PE
            for tt in range(produced_kq[0] + 1, t + 1):
                sk0 = qk_chunk_p1(b, w_sbs["wk"], 0, 2, tt)
                sk1 = qk_chunk_p1(b, w_sbs["wk"], 1, 3, tt)
                qk_chunk_p2(sk0, kT, 0, tt)
                sq0 = qk_chunk_p1(b, w_sbs["wq"], 0, 0, tt)
                qk_chunk_p2(sk1, kT, 1, tt)
                sq1 = qk_chunk_p1(b, w_sbs["wq"], 1, 1, tt)
                qk_chunk_p2(sq0, qT, 0, tt)
                qk_chunk_p2(sq1, qT, 1, tt)
                produced_kq[0] = tt

        def ensure_v(b, t):
            for tt in range(produced_v[0] + 1, t + 1):
                for tb in range(4 * tt, 4 * tt + 4):
                    v_block(b, tb)
                produced_v[0] = tt

        def needed_v(j):
            kbs = [kb for kb in range(NT) if plan[j][kb] is not None]
            return max(kb // 4 for kb in kbs) if kbs else 0

        for b in range(B):
            produced_kq[0] = -1
            produced_v[0] = -1
            for j in range(NQT):
                ensure_kq(b, j)
                load_late(b, j)
                use_mid = (j == 0 and needed_v(0) == 0)
                inject = {}
                if j + 1 < NQT:
                    # emit the next tile's K/Q chains inside this tile's
                    # attention tail: their rope latency hides under the
                    # last QK/exp/PV steps instead of stalling the j+1
                    # boundary
                    nkb = len([kb for kb in range(NT)
                               if plan[j][kb] is not None])
                    t_next = j + 1
                    if use_mid:          # flat (hp*K + idx) steps
                        ik = max(nkb + 1, 2 * nkb - 5)
                        iq = max(nkb + 2, 2 * nkb - 3)
                    else:                # per-k-block steps
                        ik = max(1, nkb - 3)
                        iq = max(2, nkb - 2)
                    inject[ik] = lambda t=t_next, bb=b: emit_k_chains(bb, t)
                    inject[iq] = lambda t=t_next, bb=b: emit_q_chains(bb, t)
                if use_mid:
                    attention_scores_mid(0, mid_hook=lambda: ensure_v(b, 0),
                                         inject=inject)
                else:
                    ensure_v(b, max(needed_v(j), j))
                    attention_scores(j, inject=inject)
                if j > 0:
                    finish_outproj(b, j - 1)
                finish_normalize(j)
            finish_outproj(b, NQT - 1)
    nc.finalize()
    return nc


def make_core_inputs(x, mask, W_qkv, b_qkv, W_proj, b_proj, mm,
                     zero_bias=False):
    """Per-core input dicts: one merged [128, NF] float32 array per core."""
    x = np.asarray(x, np.float32)
    W_qkv = np.asarray(W_qkv, np.float32)
    b_qkv = np.asarray(b_qkv, np.float32)
    W_proj = np.asarray(W_proj, np.float32)
    c2, s2p = _rope_tables()

    def fold128(w):                       # [(kt*128), m] -> [128, kt*m]
        kt = w.shape[0] // 128
        return np.ascontiguousarray(
            w.reshape(kt, 128, w.shape[1]).transpose(1, 0, 2).reshape(
                128, kt * w.shape[1]))

    def pack16(pieces):                   # bf16 [128, c] list -> f32 [128, C/2]
        cat = np.concatenate(
            [np.asarray(p, dtype=ml_dtypes.bfloat16) for p in pieces], axis=1)
        return cat.view(np.uint16).view(np.float32)

    vones = np.ones((128, NT * HPG), np.float32)
    ident = np.eye(128, dtype=np.float32)
    swp = ident[[r ^ 32 for r in range(128)]]         # XOR-32 permutation
    in_maps = []
    for c in range(NCORES):
        g = c                             # head group; all batches on core
        sl = slice(M * g, M * (g + 1))
        f32_pieces = []
        if not zero_bias:
            bq = b_qkv[0 * D:1 * D][sl]
            bk = b_qkv[1 * D:2 * D][sl]
            bv = b_qkv[2 * D:3 * D][sl]
            bqk = np.stack([bq[0:128], bq[128:256],
                            bk[0:128], bk[128:256]], axis=1)
            f32_pieces.append(bqk.astype(np.float32))
            f32_pieces.append(np.tile(bv[None, :], (128, 1)).astype(np.float32))
        b16 = pack16([
            fold128(np.ascontiguousarray(x[0].T)),
            fold128(np.ascontiguousarray(x[1].T)),
            fold128(W_qkv[:, 0 * D:1 * D][:, sl]),
            fold128(W_qkv[:, 1 * D:2 * D][:, sl]),
            fold128(W_qkv[:, 2 * D:3 * D][:, sl]),
            fold128(W_proj[sl, :]),
            vones, c2, s2p, swp, mm, ident,
        ])
        f32_pieces.append(b16)
        in_maps.append({"big": np.ascontiguousarray(
            np.concatenate(f32_pieces, axis=1), dtype=np.float32)})
    return in_maps


def gather_output(results, b_proj):
    out = np.zeros((B, T, D), np.float32)
    for c in range(NCORES):
        yc = np.asarray(results[c]["y"], dtype=np.float32)
        for b in range(B):
            out[b] += yc[b * T:(b + 1) * T]
    out += np.asarray(b_proj, np.float32)[None, None, :]
    return out


def kernel(x, mask, W_qkv, b_qkv, W_proj, b_proj):
    from concourse.bass_utils import run_bass_kernel_spmd
    plan, mm = _mask_plan(mask)
    zb = not (np.any(np.asarray(b_qkv)))
    nc = build_program(plan, mm.shape[1], zero_bias=zb)
    in_maps = make_core_inputs(x, mask, W_qkv, b_qkv, W_proj, b_proj, mm,
                               zero_bias=zb)
    res = run_bass_kernel_spmd(nc, in_maps, list(range(NCORES)))
    return gather_output(res.results, b_proj)
